# revision 7
# baseline (speedup 1.0000x reference)
"""Trainium2 Bass kernel v3 for nn_CaC_Module (dynamic-kernel dilated depthwise CNN).

Per-sample (b=8, one sample per NeuronCore):
  query = Wq @ x;  q = softmax(query over hw)   (bq cancels in softmax)
  kern  = Wk @ (x @ q^T) + bk                   (associativity trick)
  out   = x * sum_d sigmoid(depthwise3x3(x, kern, dil=d)),  d in (1,3,5)

Engine mapping (v3):
  - All 27 depthwise taps run in fp8e4 on the PE from a flat RS8=80
    padded layout. 4 of 9 taps per dilation-pass use DoubleRow pair
    matmuls (true 2x: pair stride must be 16B-aligned, hence RS8=80 and
    per-dilation pre-shifted fp8 copies for the (6,8) pair); tap 7 is a
    DVE scalar_tensor_tensor accumulating straight into the same PSUM
    group (STT is 1x-rate on this silicon, but it composes with PSUM).
  - query/softmax/G/kern GEMMs in bf16 on a separate RS=70 layout;
    softmax sums ride the exp ACT's accum_out (minus the exact junk-col
    correction), so no separate DVE reduce.
  - ScalarE: exp, e-chunk transpose drains, sigmoids over 2-window PSUM
    groups (amortizes the 352-cycle ACT ramp).
  - Merges s1+=s3; s1+=s5 on GpSimd (dense bf16), s1*=x on DVE (strided
    read stays 2x on DVE, would pay a per-row RD_CMD penalty on Q7).
  - bf16 DRAM output, host upcasts. SBUF arena overlays phase-1 buffers
    (xT, query) with phase-2 sigmoid tiles.
"""
import numpy as np
import ml_dtypes

C, H, W = 512, 64, 64
P, CB = 128, 4
RS = 70                    # bf16 flat layout row stride (query + final mul)
HEAD = 6
VPAD = 5
XLEN = 5192
NB = HEAD + VPAD * RS      # 356
NQ = H * RS                # 4480 = 35*128
NCH = NQ // P
NW = 10                    # RS70 query windows: 9 x 7 rows + 1 x 1 row
RT = 7

RS8 = 80                   # fp8 conv layout row stride (16-aligned pairs)
HEAD8 = 6
XLEN8 = 5920               # multiple of 16
NB8 = HEAD8 + VPAD * RS8   # 406
RT8 = 6
NW8 = 11                   # 10 x 6 rows + 1 x 4 rows
RATES = (1, 3, 5)
# fp8 tap pairs: verticals from variant 0; (6,8) pairs via shifted variant 1+di
PAIRS = ((0, 3), (1, 4), (2, 5))
NCORES = 8
F8 = ml_dtypes.float8_e4m3
BF16 = ml_dtypes.bfloat16
# which tap-7 units run on DVE (cb, di) -> True; rebalance knob
DVE_TAP7 = {(cb, di): True for cb in range(CB) for di in range(3)}

_CACHE = {}


def _flat8(r, x):
    return HEAD8 + (VPAD + r) * RS8 + x


def _off8(t, d):
    dy, dx = t // 3 - 1, t % 3 - 1
    return (dy * d) * RS8 + dx * d


def _win8(w):
    # (row0, nrows)
    if w < NW8 - 1:
        return RT8 * w, RT8
    return 60, 4


def _build_program():
    import bass_rust
    import concourse.bacc as bacc
    import concourse.mybir as mybir
    from concourse.tile import TileContext
    from concourse.tile import add_dep_helper

    dt = mybir.dt
    AF = mybir.ActivationFunctionType
    ALU = mybir.AluOpType
    DR = mybir.MatmulPerfMode.DoubleRow
    f32, bf16, f8, u8 = dt.float32, dt.bfloat16, dt.float8e4, dt.uint8

    nc = bacc.Bacc()
    xf_d = nc.declare_dram_parameter("xf", [C, XLEN], bf16, isOutput=False)
    x8_d = nc.declare_dram_parameter("x8", [C, 4 * XLEN8], f8, isOutput=False)
    xT_d = nc.declare_dram_parameter("xT", [NQ, C], bf16, isOutput=False)
    wkT_d = nc.declare_dram_parameter("wkT", [C, C], bf16, isOutput=False)
    wqT_d = nc.declare_dram_parameter("wqT", [C, 9], bf16, isOutput=False)
    bk_d = nc.declare_dram_parameter("bk", [C], f32, isOutput=False)
    id9h_d = nc.declare_dram_parameter("id9h", [9, 9], bf16, isOutput=False)
    id128q_d = nc.declare_dram_parameter("id128q", [P, P], f8, isOutput=False)
    out_d = nc.declare_dram_parameter("out", [C, H * W], bf16, isOutput=True)

    ARENA1 = 49152          # xT bf16 35840 (phase1) -> s1/s3/s5 x2 bufs (phase2)
    ARENA2 = 8960           # query bf16 (phase1 only)

    def pair_ap(base_ap, delta, n):
        return bass_rust.AP(
            base_ap.tensor, base_ap.offset,
            [[base_ap.ap[0][0], P], [delta, 2], [1, n]])

    with TileContext(nc) as tc:
        with (
            tc.tile_pool(name="const", bufs=1) as cpool,
            tc.tile_pool(name="diagp", bufs=2) as dpool,
            tc.tile_pool(name="ps", bufs=4, space="PSUM") as ps,
        ):
            xf = cpool.tile([P, CB, XLEN], bf16)
            x8 = cpool.tile([P, CB, 4, XLEN8], f8)
            arena1 = cpool.tile([P, ARENA1], u8)
            arena2 = cpool.tile([P, ARENA2], u8)
            wkT = cpool.tile([P, CB, C], bf16)
            wqT = cpool.tile([P, CB, 9], bf16)
            bk = cpool.tile([P, CB], f32)
            id9h = cpool.tile([9, 9], bf16)
            id128q = cpool.tile([P, P], f8)
            ssum = cpool.tile([9, NW], f32)
            stot = cpool.tile([9, 1], f32)
            stc = cpool.tile([9, 1], f32)
            rinv = cpool.tile([9, 1], f32)
            qT = cpool.tile([P, NCH, 16], bf16)
            gs = cpool.tile([9, C], bf16)
            G = cpool.tile([P, CB, 9], bf16)
            kern = cpool.tile([P, CB, 9], f32)
            ksq = cpool.tile([P, CB, 9], f32)
            km2 = cpool.tile([P, CB], f32)
            km = cpool.tile([P, CB], f32)
            kmr = cpool.tile([P, CB], f32)
            kern_sc = cpool.tile([P, CB, 9], f32)
            kern_s16 = cpool.tile([P, CB, 9], f32)
            sgs = cpool.tile([P, CB], f32)

            # ---- arena views ----
            xT = arena1[:].bitcast(bf16).rearrange("p (n c) -> p n c", c=C)
            sview = arena1[:].bitcast(bf16)
            s1v = [sview[:, 0:4096], sview[:, 4096:8192]]
            s3v = [sview[:, 8192:12288], sview[:, 12288:16384]]
            s5v = [sview[:, 16384:20480], sview[:, 20480:24576]]
            query = arena2[0:16, 0:8960].bitcast(bf16)

            # ---- input DMAs ----
            nc.sync.dma_start(out=id128q[:], in_=id128q_d[:])
            nc.sync.dma_start(out=id9h[:], in_=id9h_d[:])
            nc.sync.dma_start(
                out=wqT[:], in_=wqT_d[:].rearrange("(cb p) t -> p cb t", p=P))
            nc.sync.dma_start(
                out=bk[:], in_=bk_d[:].rearrange("(cb p) -> p cb", p=P))
            bounds = [0, 1792, 3584, XLEN]
            last = None
            for h in range(3):
                a, b = bounds[h], bounds[h + 1]
                for cb in range(CB):
                    last = nc.sync.dma_start(
                        out=xf[:, cb, a:b], in_=xf_d[cb * P:(cb + 1) * P, a:b])
            # fp8 cb0 early so PE convs can start right after kern
            d0 = nc.sync.dma_start(
                out=x8[:, 0], in_=x8_d[0:P].rearrange("p (v l) -> p v l", v=4))
            add_dep_helper(d0.ins, last.ins, reason="after xf")
            last = d0
            for h in range(3):
                a, b = 12 * h, min(12 * (h + 1), NCH)
                d1 = nc.sync.dma_start(
                    out=xT[:, a:b],
                    in_=xT_d[a * P:b * P].rearrange("(n p) c -> p n c", p=P))
                add_dep_helper(d1.ins, last.ins, reason="seq")
                last = d1
            for cb in range(1, CB):
                d2 = nc.sync.dma_start(
                    out=x8[:, cb],
                    in_=x8_d[cb * P:(cb + 1) * P].rearrange("p (v l) -> p v l", v=4))
                add_dep_helper(d2.ins, last.ins, reason="seq")
                last = d2
            d3 = nc.sync.dma_start(
                out=wkT[:], in_=wkT_d[:].rearrange("(cb p) o -> p cb o", p=P))
            add_dep_helper(d3.ins, last.ins, reason="seq")

            # ---- PE warmup (HAM clock-gate) ----
            pw = ps.tile([P, 1024], f32, tag="ps")
            for i in range(24):
                nc.tensor.matmul(pw[0:9, 0:P], lhsT=wqT[:, 0, 0:9],
                                 rhs=xf[:, 0, 0:P],
                                 start=(i == 0), stop=(i == 23))

            # ---- query windows: GEMM + exp(+accum) ----
            def qwin(w):
                if w < NW - 1:
                    return RT * w, RT, RT * RS
                return 63, 1, RS

            nxt = 0
            for w in range(NW):
                r0, nr, N = qwin(w)
                base = HEAD + (VPAD + r0) * RS
                pt = ps.tile([P, 1024], f32, tag="ps")
                psq = pt[0:9, 0:N]
                for kc in range(CB):
                    nc.tensor.matmul(
                        psq, lhsT=wqT[:, kc], rhs=xf[:, kc, base:base + N],
                        start=(kc == 0), stop=(kc == CB - 1))
                j0 = RT * w * RS
                nc.scalar.activation(query[0:9, j0:j0 + N], psq, AF.Exp,
                                     accum_out=ssum[:, w:w + 1])
                done = ((j0 + N) // P)
                while nxt < done:
                    pst = ps.tile([P, 16], bf16, tag="ps")
                    nc.tensor.transpose(
                        pst[:, 0:9], query[0:9, nxt * P:(nxt + 1) * P], id9h[:])
                    nc.vector.tensor_copy(qT[:, nxt, 0:9], pst[:, 0:9])
                    nxt += 1

            pgt = ps.tile([9, C], f32, tag="ps")
            for nch in range(NCH):
                nc.tensor.matmul(
                    pgt[:], lhsT=qT[:, nch, 0:9], rhs=xT[:, nch],
                    start=(nch == 0), stop=(nch == NCH - 1))

            nc.vector.tensor_reduce(
                stot[:], ssum[:], axis=mybir.AxisListType.X, op=ALU.add)
            # junk cols hold exp(0)=1: subtract the exact count H*(RS-W)
            nc.vector.tensor_scalar_add(stc[:], stot[:], -float(H * (RS - W)))
            nc.vector.reciprocal(rinv[:], stc[:])
            nc.vector.tensor_scalar_mul(gs[:], pgt[:], rinv[:])

            for ci in range(CB):
                psx = ps.tile([P, 16], bf16, tag="ps")
                nc.tensor.transpose(
                    psx[:, 0:9], gs[:, ci * P:(ci + 1) * P], id9h[:])
                nc.vector.tensor_copy(G[:, ci], psx[:, 0:9])

            for co in range(CB):
                psn = ps.tile([P, 16], f32, tag="ps")
                for ci in range(CB):
                    nc.tensor.matmul(
                        psn[:, 0:9],
                        lhsT=wkT[:, ci, co * P:(co + 1) * P], rhs=G[:, ci],
                        start=(ci == 0), stop=(ci == CB - 1))
                nc.vector.tensor_scalar_add(kern[:, co], psn[:, 0:9],
                                            bk[:, co:co + 1])

            # per-channel fp8 scaling: s_c = 64/max|kern_row|, compensated
            # exactly by the sigmoid's per-partition scale (m/1024).
            nc.scalar.square(ksq[:].rearrange("p cb t -> p (cb t)"),
                             kern[:].rearrange("p cb t -> p (cb t)"))
            nc.vector.tensor_reduce(
                km2[:].rearrange("p (cb o) -> p cb o", o=1), ksq[:],
                axis=mybir.AxisListType.X, op=ALU.max)
            nc.scalar.activation(km[:], km2[:], AF.Sqrt)
            nc.vector.reciprocal(kmr[:], km[:])
            for co in range(CB):
                nc.vector.tensor_scalar_mul(kern_sc[:, co], kern[:, co],
                                            kmr[:, co:co + 1])
            nc.vector.tensor_scalar_mul(
                kern_s16[:].rearrange("p cb t -> p (cb t)"),
                kern_sc[:].rearrange("p cb t -> p (cb t)"), 64.0)
            nc.vector.tensor_scalar_mul(sgs[:], km[:], 1.0 / 1024.0)

            # ---- phase 2: all-fp8 convs on PE (+ tap7 via DVE STT) ----
            GROUPS = [(0, 1), (2, 3), (4, 5), (6, 7), (8, 9), (10,)]
            for cb in range(CB):
                # diags: 3 vertical pairs, the (6,8) pair, tap-7/center single
                dg8 = dpool.tile([P, 4, 2, P], f8, tag="dg8")
                dg7 = dpool.tile([P, P], f8, tag="dg7")
                for pi, (ta, tb) in enumerate(PAIRS):
                    nc.vector.tensor_scalar_mul(
                        dg8[:, pi, 0], id128q[:], kern_sc[:, cb, ta:ta + 1])
                    nc.vector.tensor_scalar_mul(
                        dg8[:, pi, 1], id128q[:], kern_sc[:, cb, tb:tb + 1])
                nc.vector.tensor_scalar_mul(
                    dg8[:, 3, 0], id128q[:], kern_sc[:, cb, 6:7])
                nc.vector.tensor_scalar_mul(
                    dg8[:, 3, 1], id128q[:], kern_sc[:, cb, 8:9])
                nc.vector.tensor_scalar_mul(dg7[:], id128q[:], kern_sc[:, cb, 7:8])

                s_of = {0: s1v[cb % 2], 1: s3v[cb % 2], 2: s5v[cb % 2]}

                for dgrp in ((0,), (1,), (2,)):
                    for g, grp in enumerate(GROUPS):
                        pd_a = ps.tile([P, 1024], f32, tag="ps")
                        pds = {dgrp[0]: pd_a}
                        if len(dgrp) > 1:
                            pd_b = ps.tile([P, 1024], f32, tag="ps")
                            pds[dgrp[1]] = pd_b
                        for pi in range(4):
                            for di in dgrp:
                                d = RATES[di]
                                for wi, w in enumerate(grp):
                                    r0, nr = _win8(w)
                                    N = nr * RS8
                                    if pi < 3:
                                        ta = PAIRS[pi][0]
                                        oa = _flat8(r0, 0) + _off8(ta, d)
                                        delta = (_off8(PAIRS[pi][1], d)
                                                 - _off8(ta, d))
                                        a0 = x8[:, cb, 0, oa:oa + N]
                                    else:
                                        # (6,8): A from variant 0, B from 1+di
                                        oa = _flat8(r0, 0) + _off8(6, d)
                                        delta = (1 + di) * XLEN8 + 16
                                        a0 = x8[:, cb, 0, oa:oa + N]
                                    nc.tensor.matmul(
                                        pds[di][:, wi * 512:wi * 512 + N],
                                        lhsT=dg8[:, pi],
                                        rhs=pair_ap(a0, delta, N),
                                        start=(pi == 0),
                                        stop=(pi == 3 and DVE_TAP7[(cb, di)]),
                                        perf_mode=DR)
                        # tap 7 (dy=+1, dx=0): DVE STT into the PSUM group,
                        # or PE single matmul (stop closes the group)
                        for di in dgrp:
                            d = RATES[di]
                            for wi, w in enumerate(grp):
                                r0, nr = _win8(w)
                                N = nr * RS8
                                o7 = _flat8(r0, 0) + _off8(7, d)
                                slot = pds[di][:, wi * 512:wi * 512 + N]
                                if DVE_TAP7[(cb, di)]:
                                    nc.vector.scalar_tensor_tensor(
                                        slot,
                                        in0=x8[:, cb, 0, o7:o7 + N],
                                        scalar=kern_s16[:, cb, 7:8],
                                        in1=slot,
                                        op0=ALU.mult, op1=ALU.add)
                                else:
                                    nc.tensor.matmul(
                                        slot, lhsT=dg7[:],
                                        rhs=x8[:, cb, 0, o7:o7 + N],
                                        start=False, stop=True)
                        # sigmoid per (dil, group)
                        for di in dgrp:
                            s = s_of[di]
                            if len(grp) == 2:
                                nc.scalar.activation(
                                    s[:, g * 2 * RT8 * W:(g + 1) * 2 * RT8 * W]
                                    .rearrange("p (u r c) -> p u r c", u=2, c=W),
                                    pds[di][:]
                                    .rearrange("p (u n) -> p u n", u=2)
                                    [:, :, 0:RT8 * RS8]
                                    .rearrange("p u (r c) -> p u r c", c=RS8)
                                    [:, :, :, 0:W],
                                    AF.Sigmoid, scale=sgs[:, cb:cb + 1])
                            else:
                                nc.scalar.activation(
                                    s[:, 60 * W:64 * W]
                                    .rearrange("p (r c) -> p r c", c=W),
                                    pds[di][:, 0:4 * RS8]
                                    .rearrange("p (r c) -> p r c", c=RS8)
                                    [:, :, 0:W],
                                    AF.Sigmoid, scale=sgs[:, cb:cb + 1])

                # ---- merge (in place into s1) + store ----
                s1, s3, s5 = s_of[0], s_of[1], s_of[2]
                nc.gpsimd.tensor_add(s1[:], s1[:], s3[:])
                nc.gpsimd.tensor_add(s1[:], s1[:], s5[:])
                nc.vector.tensor_mul(
                    s1[:].rearrange("p (r c) -> p r c", c=W),
                    s1[:].rearrange("p (r c) -> p r c", c=W),
                    xf[:, cb, NB:NB + NQ]
                    .rearrange("p (r c) -> p r c", c=RS)[:, :, 0:W])
                nc.sync.dma_start(
                    out=out_d[cb * P:(cb + 1) * P, 0:H * W // 2],
                    in_=s1[:, 0:H * W // 2])
                nc.sync.dma_start(
                    out=out_d[cb * P:(cb + 1) * P, H * W // 2:H * W],
                    in_=s1[:, H * W // 2:H * W])
    nc.finalize()
    return nc


def _get_program():
    if "nc" not in _CACHE:
        _CACHE["nc"] = _build_program()
    return _CACHE["nc"]


def make_in_maps(x, Wk, bk, Wq, bq=None):
    x = np.ascontiguousarray(np.asarray(x, dtype=np.float32))
    B = x.shape[0]
    assert B == NCORES and x.shape[1:] == (C, H, W)
    xf = np.zeros((B, C, XLEN), dtype=BF16)
    view = xf[:, :, HEAD:HEAD + (H + 2 * VPAD) * RS].reshape(
        B, C, H + 2 * VPAD, RS)
    view[:, :, VPAD:VPAD + H, 0:W] = x.astype(BF16)
    # fp8 RS8=80 layout, 4 variants: base + shifted by 16-2d for d in (1,3,5)
    x80 = np.zeros((B, C, XLEN8), dtype=F8)
    v80 = x80[:, :, HEAD8:HEAD8 + (H + 2 * VPAD) * RS8].reshape(
        B, C, H + 2 * VPAD, RS8)
    v80[:, :, VPAD:VPAD + H, 0:W] = (16.0 * x).astype(F8)
    x8 = np.zeros((B, C, 4, XLEN8), dtype=F8)
    x8[:, :, 0] = x80
    for vi, d in enumerate(RATES):
        s = 16 - 2 * d
        x8[:, :, 1 + vi, s:] = x80[:, :, :XLEN8 - s]
    xT = np.ascontiguousarray(
        np.swapaxes(xf[:, :, NB:NB + NQ].astype(np.float32), 1, 2)).astype(BF16)
    shared = {
        "wkT": np.ascontiguousarray(np.asarray(Wk, np.float32).T).astype(BF16),
        "wqT": np.ascontiguousarray(np.asarray(Wq, np.float32).T).astype(BF16),
        "bk": np.ascontiguousarray(np.asarray(bk, np.float32)),
        "id9h": np.eye(9).astype(BF16),
        "id128q": (64.0 * np.eye(P)).astype(F8),
    }
    return [dict(shared, xf=np.ascontiguousarray(xf[i]),
                 x8=np.ascontiguousarray(x8[i].reshape(C, 4 * XLEN8)),
                 xT=xT[i])
            for i in range(B)]


def kernel(x, Wk, bk, Wq, bq):
    from concourse.bass_utils import run_bass_kernel_spmd

    in_maps = make_in_maps(x, Wk, bk, Wq, bq)
    nc = _get_program()
    res = run_bass_kernel_spmd(nc, in_maps, list(range(NCORES))).results
    return np.stack([res[i]["out"] for i in range(NCORES)]).astype(
        np.float32).reshape(NCORES, C, H, W)


# revision 10
# speedup vs baseline: 1.1788x; 1.1788x over previous
"""Trainium2 Bass kernel v3 for nn_CaC_Module (dynamic-kernel dilated depthwise CNN).

Per-sample (b=8, one sample per NeuronCore):
  query = Wq @ x;  q = softmax(query over hw)   (bq cancels in softmax)
  kern  = Wk @ (x @ q^T) + bk                   (associativity trick)
  out   = x * sum_d sigmoid(depthwise3x3(x, kern, dil=d)),  d in (1,3,5)

Engine mapping (v3):
  - All 27 depthwise taps run in fp8e4 on the PE from a flat RS8=80
    padded layout. 4 of 9 taps per dilation-pass use DoubleRow pair
    matmuls (true 2x: pair stride must be 16B-aligned, hence RS8=80 and
    per-dilation pre-shifted fp8 copies for the (6,8) pair); tap 7 is a
    DVE scalar_tensor_tensor accumulating straight into the same PSUM
    group (STT is 1x-rate on this silicon, but it composes with PSUM).
  - query/softmax/G/kern GEMMs in bf16 on a separate RS=70 layout;
    softmax sums ride the exp ACT's accum_out (minus the exact junk-col
    correction), so no separate DVE reduce.
  - ScalarE: exp, e-chunk transpose drains, sigmoids over 2-window PSUM
    groups (amortizes the 352-cycle ACT ramp).
  - Merges s1+=s3; s1+=s5 on GpSimd (dense bf16), s1*=x on DVE (strided
    read stays 2x on DVE, would pay a per-row RD_CMD penalty on Q7).
  - bf16 DRAM output, host upcasts. SBUF arena overlays phase-1 buffers
    (xT, query) with phase-2 sigmoid tiles.
"""
import numpy as np
import ml_dtypes

C, H, W = 512, 64, 64
P, CB = 128, 4
RS = 70                    # bf16 flat layout row stride (query + final mul)
HEAD = 6
VPAD = 5
XLEN = 5200
NB = HEAD + VPAD * RS      # 356
NQ = H * RS                # 4480 = 35*128
NCH = NQ // P
NW = 10                    # RS70 query windows: 9 x 7 rows + 1 x 1 row
RT = 7

RS8 = 80                   # fp8 conv layout row stride (16-aligned pairs)
HEAD8 = 6
XLEN8 = 5920               # multiple of 16
NB8 = HEAD8 + VPAD * RS8   # 406
RT8 = 6
NW8 = 11                   # 10 x 6 rows + 1 x 4 rows
RATES = (1, 3, 5)
# fp8 tap pairs: verticals from variant 0; (6,8) pairs via shifted variant 1+di
PAIRS = ((0, 3), (1, 4), (2, 5))
NCORES = 8
F8 = ml_dtypes.float8_e4m3
BF16 = ml_dtypes.bfloat16
# which tap-7 units run on DVE (cb, di) -> True; rebalance knob
DVE_TAP7 = {(cb, di): True for cb in range(CB) for di in range(3)}

_CACHE = {}


def _flat8(r, x):
    return HEAD8 + (VPAD + r) * RS8 + x


def _off8(t, d):
    dy, dx = t // 3 - 1, t % 3 - 1
    return (dy * d) * RS8 + dx * d


def _win8(w):
    # (row0, nrows)
    if w < NW8 - 1:
        return RT8 * w, RT8
    return 60, 4


def _build_program():
    import bass_rust
    import concourse.bacc as bacc
    import concourse.mybir as mybir
    from concourse.tile import TileContext
    from concourse.tile import add_dep_helper

    dt = mybir.dt
    AF = mybir.ActivationFunctionType
    ALU = mybir.AluOpType
    DR = mybir.MatmulPerfMode.DoubleRow
    f32, bf16, f8, u8 = dt.float32, dt.bfloat16, dt.float8e4, dt.uint8

    nc = bacc.Bacc()
    xf_d = nc.declare_dram_parameter("xf", [C, XLEN], bf16, isOutput=False)
    x8_d = nc.declare_dram_parameter("x8", [C, 4 * XLEN8], f8, isOutput=False)
    xT_d = nc.declare_dram_parameter("xT", [NQ, C], bf16, isOutput=False)
    wkT_d = nc.declare_dram_parameter("wkT", [C, C], bf16, isOutput=False)
    wqT_d = nc.declare_dram_parameter("wqT", [C, 9], bf16, isOutput=False)
    bk_d = nc.declare_dram_parameter("bk", [C], f32, isOutput=False)
    id9h_d = nc.declare_dram_parameter("id9h", [9, 9], bf16, isOutput=False)
    id128q_d = nc.declare_dram_parameter("id128q", [P, P], f8, isOutput=False)
    out_d = nc.declare_dram_parameter("out", [C, H * W], bf16, isOutput=True)

    ARENA1 = 49152          # xT bf16 35840 (phase1) -> s1/s3/s5 x2 bufs (phase2)
    ARENA2 = 8960           # query bf16 (phase1 only)

    def pair_ap(base_ap, delta, n):
        return bass_rust.AP(
            base_ap.tensor, base_ap.offset,
            [[base_ap.ap[0][0], P], [delta, 2], [1, n]])

    with TileContext(nc) as tc:
        with (
            tc.tile_pool(name="const", bufs=1) as cpool,
            tc.tile_pool(name="diagp", bufs=2) as dpool,
            tc.tile_pool(name="ps", bufs=4, space="PSUM") as ps,
        ):
            xf = cpool.tile([P, CB, XLEN], bf16)
            x8 = cpool.tile([P, CB, 4, XLEN8], f8)
            arena1 = cpool.tile([P, ARENA1], u8)
            arena2 = cpool.tile([P, ARENA2], u8)
            wkT = cpool.tile([P, CB, C], bf16)
            wqT = cpool.tile([P, CB, 9], bf16)
            bk = cpool.tile([P, CB], f32)
            id9h = cpool.tile([9, 9], bf16)
            id128q = cpool.tile([P, P], f8)
            ssum = cpool.tile([9, NW], f32)
            stot = cpool.tile([9, 1], f32)
            stc = cpool.tile([9, 1], f32)
            rinv = cpool.tile([9, 1], f32)
            qT = cpool.tile([P, NCH, 16], bf16)
            gs = cpool.tile([9, C], bf16)
            G = cpool.tile([P, CB, 9], bf16)
            kern = cpool.tile([P, CB, 9], f32)
            ksq = cpool.tile([P, CB, 9], f32)
            km2 = cpool.tile([P, CB], f32)
            km = cpool.tile([P, CB], f32)
            kmr = cpool.tile([P, CB], f32)
            kern_sc = cpool.tile([P, CB, 9], f32)
            kern_s16 = cpool.tile([P, CB, 9], f32)
            sgs = cpool.tile([P, CB], f32)

            # ---- arena views ----
            xT = arena1[:].bitcast(bf16).rearrange("p (n c) -> p n c", c=C)
            sview = arena1[:].bitcast(bf16)
            s1v = [sview[:, 0:4096], sview[:, 4096:8192]]
            s3v = [sview[:, 8192:12288], sview[:, 12288:16384]]
            s5v = [sview[:, 16384:20480], sview[:, 20480:24576]]
            query = arena2[0:16, 0:8960].bitcast(bf16)

            # ---- input DMAs ----
            nc.sync.dma_start(out=id128q[:], in_=id128q_d[:])
            nc.sync.dma_start(out=id9h[:], in_=id9h_d[:])
            nc.sync.dma_start(
                out=wqT[:], in_=wqT_d[:].rearrange("(cb p) t -> p cb t", p=P))
            nc.sync.dma_start(
                out=bk[:], in_=bk_d[:].rearrange("(cb p) -> p cb", p=P))
            bounds = [0, 1792, 3584, XLEN]
            last = None
            for h in range(3):
                a, b = bounds[h], bounds[h + 1]
                for cb in range(CB):
                    last = nc.sync.dma_start(
                        out=xf[:, cb, a:b], in_=xf_d[cb * P:(cb + 1) * P, a:b])
            # After xf: xT chunks + fp8 cb0 + wkT in PARALLEL across queues
            # (each gated only on the last xf piece, not on each other).
            stage2 = []
            for h in range(6):
                a, b = 6 * h, min(6 * (h + 1), NCH)
                d1 = nc.sync.dma_start(
                    out=xT[:, a:b],
                    in_=xT_d[a * P:b * P].rearrange("(n p) c -> p n c", p=P))
                add_dep_helper(d1.ins, last.ins, reason="after xf")
                stage2.append(d1)
            d0 = nc.sync.dma_start(
                out=x8[:, 0], in_=x8_d[0:P].rearrange("p (v l) -> p v l", v=4))
            add_dep_helper(d0.ins, last.ins, reason="after xf")
            stage2.append(d0)
            d3 = nc.sync.dma_start(
                out=wkT[:], in_=wkT_d[:].rearrange("(cb p) o -> p cb o", p=P))
            add_dep_helper(d3.ins, last.ins, reason="after xf")
            stage2.append(d3)
            # remaining fp8 blocks trail the phase-1-critical loads
            for cb in range(1, CB):
                d2 = nc.sync.dma_start(
                    out=x8[:, cb],
                    in_=x8_d[cb * P:(cb + 1) * P].rearrange("p (v l) -> p v l", v=4))
                add_dep_helper(d2.ins, stage2[-3 + (cb - 1)].ins, reason="seq")

            # ---- PE warmup (HAM clock-gate) ----
            pw = ps.tile([P, 1024], f32, tag="ps")
            for i in range(24):
                nc.tensor.matmul(pw[0:9, 0:P], lhsT=wqT[:, 0, 0:9],
                                 rhs=xf[:, 0, 0:P],
                                 start=(i == 0), stop=(i == 23))

            # ---- query windows: GEMM + exp(+accum) ----
            def qwin(w):
                if w < NW - 1:
                    return RT * w, RT, RT * RS
                return 63, 1, RS

            nxt = 0
            for w in range(NW):
                r0, nr, N = qwin(w)
                base = HEAD + (VPAD + r0) * RS
                pt = ps.tile([P, 1024], f32, tag="ps")
                psq = pt[0:9, 0:N]
                for kc in range(CB):
                    nc.tensor.matmul(
                        psq, lhsT=wqT[:, kc], rhs=xf[:, kc, base:base + N],
                        start=(kc == 0), stop=(kc == CB - 1))
                j0 = RT * w * RS
                nc.scalar.activation(query[0:9, j0:j0 + N], psq, AF.Exp,
                                     accum_out=ssum[:, w:w + 1])
                done = ((j0 + N) // P)
                while nxt < done:
                    pst = ps.tile([P, 16], bf16, tag="ps")
                    nc.tensor.transpose(
                        pst[:, 0:9], query[0:9, nxt * P:(nxt + 1) * P], id9h[:])
                    nc.vector.tensor_copy(qT[:, nxt, 0:9], pst[:, 0:9])
                    nxt += 1

            pgt = ps.tile([9, C], f32, tag="ps")
            for nch in range(NCH):
                nc.tensor.matmul(
                    pgt[:], lhsT=qT[:, nch, 0:9], rhs=xT[:, nch],
                    start=(nch == 0), stop=(nch == NCH - 1))

            nc.vector.tensor_reduce(
                stot[:], ssum[:], axis=mybir.AxisListType.X, op=ALU.add)
            # junk cols hold exp(0)=1: subtract the exact count H*(RS-W)
            nc.vector.tensor_scalar_add(stc[:], stot[:], -float(H * (RS - W)))
            nc.vector.reciprocal(rinv[:], stc[:])
            nc.vector.tensor_scalar_mul(gs[:], pgt[:], rinv[:])

            for ci in range(CB):
                psx = ps.tile([P, 16], bf16, tag="ps")
                nc.tensor.transpose(
                    psx[:, 0:9], gs[:, ci * P:(ci + 1) * P], id9h[:])
                nc.vector.tensor_copy(G[:, ci], psx[:, 0:9])

            for co in range(CB):
                psn = ps.tile([P, 16], f32, tag="ps")
                for ci in range(CB):
                    nc.tensor.matmul(
                        psn[:, 0:9],
                        lhsT=wkT[:, ci, co * P:(co + 1) * P], rhs=G[:, ci],
                        start=(ci == 0), stop=(ci == CB - 1))
                nc.vector.tensor_scalar_add(kern[:, co], psn[:, 0:9],
                                            bk[:, co:co + 1])

            # per-channel fp8 scaling: s_c = 64/max|kern_row|, compensated
            # exactly by the sigmoid's per-partition scale (m/1024).
            nc.scalar.square(ksq[:].rearrange("p cb t -> p (cb t)"),
                             kern[:].rearrange("p cb t -> p (cb t)"))
            nc.vector.tensor_reduce(
                km2[:].rearrange("p (cb o) -> p cb o", o=1), ksq[:],
                axis=mybir.AxisListType.X, op=ALU.max)
            nc.scalar.activation(km[:], km2[:], AF.Sqrt)
            nc.vector.reciprocal(kmr[:], km[:])
            for co in range(CB):
                nc.vector.tensor_scalar_mul(kern_sc[:, co], kern[:, co],
                                            kmr[:, co:co + 1])
            nc.vector.tensor_scalar_mul(
                kern_s16[:].rearrange("p cb t -> p (cb t)"),
                kern_sc[:].rearrange("p cb t -> p (cb t)"), 1024.0)
            nc.vector.tensor_scalar_mul(sgs[:], km[:], 1.0 / 1024.0)

            # ---- phase 2: all-fp8 convs on PE (+ tap7 via DVE STT) ----
            GROUPS = [(0, 1), (2, 3), (4, 5), (6, 7), (8, 9), (10,)]
            for cb in range(CB):
                # diags: 3 vertical pairs, the (6,8) pair, tap-7/center single
                dg8 = dpool.tile([P, 4, 2, P], f8, tag="dg8")
                dg7 = dpool.tile([P, P], f8, tag="dg7")
                for pi, (ta, tb) in enumerate(PAIRS):
                    nc.vector.tensor_scalar_mul(
                        dg8[:, pi, 0], id128q[:], kern_sc[:, cb, ta:ta + 1])
                    nc.vector.tensor_scalar_mul(
                        dg8[:, pi, 1], id128q[:], kern_sc[:, cb, tb:tb + 1])
                nc.vector.tensor_scalar_mul(
                    dg8[:, 3, 0], id128q[:], kern_sc[:, cb, 6:7])
                nc.vector.tensor_scalar_mul(
                    dg8[:, 3, 1], id128q[:], kern_sc[:, cb, 8:9])
                nc.vector.tensor_scalar_mul(dg7[:], id128q[:], kern_sc[:, cb, 7:8])

                s_of = {0: s1v[cb % 2], 1: s3v[cb % 2], 2: s5v[cb % 2]}

                HALVES = [((0, 1), (2, 3), (4, 5)), ((6, 7), (8, 9), (10,))]
                for di in range(3):
                    d = RATES[di]
                    for hi, half in enumerate(HALVES):
                        tiles = []
                        slots = {}
                        for grp in half:
                            pd_t = ps.tile([P, 1024], f32, tag="ps")
                            tiles.append((grp, pd_t))
                            for wi, w in enumerate(grp):
                                r0, nr = _win8(w)
                                slots[w] = pd_t[:, wi * 512:wi * 512 + nr * RS8]
                        ws = [w for grp in half for w in grp]
                        # 4 fp8 DoubleRow pairs, one LDWEIGHTS per 5-6 matmuls
                        for pi in range(4):
                            for w in ws:
                                r0, nr = _win8(w)
                                N = nr * RS8
                                if pi < 3:
                                    ta = PAIRS[pi][0]
                                    oa = _flat8(r0, 0) + _off8(ta, d)
                                    delta = (_off8(PAIRS[pi][1], d)
                                             - _off8(ta, d))
                                else:
                                    # (6,8): A from variant 0, B from 1+di
                                    oa = _flat8(r0, 0) + _off8(6, d)
                                    delta = (1 + di) * XLEN8 + 16
                                a0 = x8[:, cb, 0, oa:oa + N]
                                nc.tensor.matmul(
                                    slots[w], lhsT=dg8[:, pi],
                                    rhs=pair_ap(a0, delta, N),
                                    start=(pi == 0), stop=(pi == 3),
                                    perf_mode=DR)
                        # tap 7 (dy=+1, dx=0): DVE STT of exact bf16 x into
                        # the PSUM group via an RS70->RS8 remapping AP (the
                        # 10 wrap cols per row land in PSUM junk cols)
                        for w in ws:
                            r0, nr = _win8(w)
                            xb = xf[:, cb, 0:1]
                            base = (xb.offset + NB + (r0 + d) * RS)
                            in0 = bass_rust.AP(
                                xb.tensor, base,
                                [[xb.ap[0][0], P], [RS, nr], [1, RS8]])
                            nc.vector.scalar_tensor_tensor(
                                slots[w].rearrange("p (r c) -> p r c", c=RS8),
                                in0=in0,
                                scalar=kern_s16[:, cb, 7:8],
                                in1=slots[w].rearrange("p (r c) -> p r c", c=RS8),
                                op0=ALU.mult, op1=ALU.add)
                        # sigmoid per psum tile
                        s = s_of[di]
                        for grp, pd_t in tiles:
                            if len(grp) == 2:
                                g = grp[0] // 2
                                nc.scalar.activation(
                                    s[:, g * 2 * RT8 * W:(g + 1) * 2 * RT8 * W]
                                    .rearrange("p (u r c) -> p u r c", u=2, c=W),
                                    pd_t[:]
                                    .rearrange("p (u n) -> p u n", u=2)
                                    [:, :, 0:RT8 * RS8]
                                    .rearrange("p u (r c) -> p u r c", c=RS8)
                                    [:, :, :, 0:W],
                                    AF.Sigmoid, scale=sgs[:, cb:cb + 1])
                            else:
                                nc.scalar.activation(
                                    s[:, 60 * W:64 * W]
                                    .rearrange("p (r c) -> p r c", c=W),
                                    pd_t[:, 0:4 * RS8]
                                    .rearrange("p (r c) -> p r c", c=RS8)
                                    [:, :, 0:W],
                                    AF.Sigmoid, scale=sgs[:, cb:cb + 1])

                # ---- merge (in place into s1) + store ----
                s1, s3, s5 = s_of[0], s_of[1], s_of[2]
                nc.gpsimd.tensor_add(s1[:], s1[:], s3[:])
                nc.gpsimd.tensor_add(s1[:], s1[:], s5[:])
                nc.vector.tensor_mul(
                    s1[:].rearrange("p (r c) -> p r c", c=W),
                    s1[:].rearrange("p (r c) -> p r c", c=W),
                    xf[:, cb, NB:NB + NQ]
                    .rearrange("p (r c) -> p r c", c=RS)[:, :, 0:W])
                nc.sync.dma_start(
                    out=out_d[cb * P:(cb + 1) * P, 0:H * W // 2],
                    in_=s1[:, 0:H * W // 2])
                nc.sync.dma_start(
                    out=out_d[cb * P:(cb + 1) * P, H * W // 2:H * W],
                    in_=s1[:, H * W // 2:H * W])
    nc.finalize()
    return nc


def _get_program():
    if "nc" not in _CACHE:
        _CACHE["nc"] = _build_program()
    return _CACHE["nc"]


def make_in_maps(x, Wk, bk, Wq, bq=None):
    x = np.ascontiguousarray(np.asarray(x, dtype=np.float32))
    B = x.shape[0]
    assert B == NCORES and x.shape[1:] == (C, H, W)
    xf = np.zeros((B, C, XLEN), dtype=BF16)
    view = xf[:, :, HEAD:HEAD + (H + 2 * VPAD) * RS].reshape(
        B, C, H + 2 * VPAD, RS)
    view[:, :, VPAD:VPAD + H, 0:W] = x.astype(BF16)
    # fp8 RS8=80 layout, 4 variants: base + shifted by 16-2d for d in (1,3,5)
    x80 = np.zeros((B, C, XLEN8), dtype=F8)
    v80 = x80[:, :, HEAD8:HEAD8 + (H + 2 * VPAD) * RS8].reshape(
        B, C, H + 2 * VPAD, RS8)
    v80[:, :, VPAD:VPAD + H, 0:W] = (16.0 * x).astype(F8)
    x8 = np.zeros((B, C, 4, XLEN8), dtype=F8)
    x8[:, :, 0] = x80
    for vi, d in enumerate(RATES):
        s = 16 - 2 * d
        x8[:, :, 1 + vi, s:] = x80[:, :, :XLEN8 - s]
    xT = np.ascontiguousarray(
        np.swapaxes(xf[:, :, NB:NB + NQ].astype(np.float32), 1, 2)).astype(BF16)
    shared = {
        "wkT": np.ascontiguousarray(np.asarray(Wk, np.float32).T).astype(BF16),
        "wqT": np.ascontiguousarray(np.asarray(Wq, np.float32).T).astype(BF16),
        "bk": np.ascontiguousarray(np.asarray(bk, np.float32)),
        "id9h": np.eye(9).astype(BF16),
        "id128q": (64.0 * np.eye(P)).astype(F8),
    }
    return [dict(shared, xf=np.ascontiguousarray(xf[i]),
                 x8=np.ascontiguousarray(x8[i].reshape(C, 4 * XLEN8)),
                 xT=xT[i])
            for i in range(B)]


def kernel(x, Wk, bk, Wq, bq):
    from concourse.bass_utils import run_bass_kernel_spmd

    in_maps = make_in_maps(x, Wk, bk, Wq, bq)
    nc = _get_program()
    res = run_bass_kernel_spmd(nc, in_maps, list(range(NCORES))).results
    return np.stack([res[i]["out"] for i in range(NCORES)]).astype(
        np.float32).reshape(NCORES, C, H, W)
